# revision 5
# baseline (speedup 1.0000x reference)
"""Trainium2 Bass kernel for a local-attention transformer block.

Problem: x(4,4096,1024) -> LN1 -> qkv(16 heads, d=64) -> local attention
(window 128, look +-1 block) -> proj -> +residual -> LN2 -> MLP(4096, exact
gelu) -> +residual.

Sharding: 8 cores x 2048 tokens (half a sequence each). Odd cores receive
their tokens REVERSED on the host so that every core sees the identical
geometry (the edge-masked attention block is always local block 0, the valid
halo block is always on the right at local block 16). Local attention with a
symmetric +-1-block window is exactly equivariant under token reversal, so
the program is fully SPMD-uniform: no masks, no per-core control flow.
Host reverses odd-core outputs back and concatenates.
"""

import numpy as np

import concourse.bass as bass
import concourse.bacc as bacc
import concourse.mybir as mybir
import concourse.tile as tile
from concourse import bass_utils
from concourse.masks import make_identity

F32 = mybir.dt.float32

B, N, DIM = 4, 4096, 1024
HEADS, DFF, WIN = 16, 4096, 128
HD = DIM // HEADS  # 64
EPS = 1e-5
NCORES = 8
TOK = 2048           # own tokens per core
TOKH = TOK + WIN     # 2176 incl. right halo block
NBLK = TOK // WIN    # 16 query blocks per core
SCALE = HD ** -0.5
NC_DIM = DIM // 128   # 8 c-chunks
NC_FF = DFF // 128    # 32 f-chunks

# matmul input dtype knobs (float32 | float32r | bfloat16-as-storage is not
# done here; float32r is a bitcast so data stays fp32 in SBUF)
MM_BIG = mybir.dt.float32     # qkv / proj / fc1 / fc2
MM_ATT = mybir.dt.float32     # attention sim / pv


def _mm_cast(ap, dt):
    return ap if dt == F32 else ap.bitcast(dt)


def _layernorm_tile(nc, pool, x_t, eps_tile):
    """x_t: SBUF [128, DIM] fp32 -> returns (rstd[128,1], negmurstd[128,1])."""
    stats = pool.tile([128, 2, 6], F32, tag="ln_stats")
    nc.vector.bn_stats(out=stats[:, 0, :], in_=x_t[:, 0:512])
    nc.vector.bn_stats(out=stats[:, 1, :], in_=x_t[:, 512:1024])
    mv = pool.tile([128, 2], F32, tag="ln_mv")
    nc.vector.bn_aggr(out=mv[:], in_=stats[:])
    rstd = pool.tile([128, 1], F32, tag="ln_rstd")
    nc.scalar.activation(out=rstd[:], in_=mv[:, 1:2],
                         func=mybir.ActivationFunctionType.Sqrt,
                         bias=eps_tile[:], scale=1.0)
    nc.vector.reciprocal(out=rstd[:], in_=rstd[:])
    nmr = pool.tile([128, 1], F32, tag="ln_nmr")
    # nmr = -(mu * rstd)
    nc.vector.tensor_scalar(out=nmr[:], in0=mv[:, 0:1], scalar1=rstd[:],
                            op0=mybir.AluOpType.mult,
                            scalar2=-1.0, op1=mybir.AluOpType.mult)
    return rstd, nmr


def _build_program():
    nc = bacc.Bacc("TRN2", target_bir_lowering=False, debug=False,
                   num_devices=NCORES)

    # ---- I/O ----
    x_loc = nc.dram_tensor("x_loc", [TOKH, DIM], F32, kind="ExternalInput").ap()
    ln1_w = nc.dram_tensor("ln1_w", [DIM], F32, kind="ExternalInput").ap()
    ln1_b = nc.dram_tensor("ln1_b", [DIM], F32, kind="ExternalInput").ap()
    ln2_w = nc.dram_tensor("ln2_w", [DIM], F32, kind="ExternalInput").ap()
    ln2_b = nc.dram_tensor("ln2_b", [DIM], F32, kind="ExternalInput").ap()
    wqkT = nc.dram_tensor("wqkT", [DIM, 2 * DIM], F32, kind="ExternalInput").ap()
    bqk = nc.dram_tensor("bqk", [2 * DIM], F32, kind="ExternalInput").ap()
    wvT = nc.dram_tensor("wvT", [DIM, DIM], F32, kind="ExternalInput").ap()
    bv = nc.dram_tensor("bv", [DIM], F32, kind="ExternalInput").ap()
    wprojT = nc.dram_tensor("wprojT", [DIM, DIM], F32, kind="ExternalInput").ap()
    bproj = nc.dram_tensor("bproj", [DIM], F32, kind="ExternalInput").ap()
    wfc1T = nc.dram_tensor("wfc1T", [DIM, DFF], F32, kind="ExternalInput").ap()
    bfc1 = nc.dram_tensor("bfc1", [DFF], F32, kind="ExternalInput").ap()
    wfc2T = nc.dram_tensor("wfc2T", [DFF, DIM], F32, kind="ExternalInput").ap()
    bfc2 = nc.dram_tensor("bfc2", [DIM], F32, kind="ExternalInput").ap()
    out_loc = nc.dram_tensor("out_loc", [TOK, DIM], F32, kind="ExternalOutput").ap()

    NT_H = TOKH // 128   # 17 token tiles incl halo
    NT = TOK // 128      # 16 own token tiles

    with tile.TileContext(nc) as tc:
        with (
            tc.tile_pool(name="dram", bufs=1, space="DRAM") as dpool,
            tc.tile_pool(name="consts", bufs=1) as cpool,
        ):
            # ---- DRAM scratch ----
            d_xnT = dpool.tile([DIM, TOKH], F32)      # LN1 out, transposed
            d_qT = dpool.tile([DIM, TOK], F32)        # q (prescaled), transposed
            d_kT = dpool.tile([DIM, TOKH], F32)
            d_v = dpool.tile([TOKH, DIM], F32)        # token-major
            d_attnT = dpool.tile([DIM, TOK], F32)
            d_x1 = dpool.tile([TOK, DIM], F32)        # post-attn residual
            d_x1nT = dpool.tile([DIM, TOK], F32)      # LN2 out, transposed
            d_gT = dpool.tile([DFF, TOK], F32)        # gelu out, transposed

            # ---- constants ----
            ident = cpool.tile([128, 128], F32)
            make_identity(nc, ident[:])
            eps_t = cpool.tile([128, 1], F32)
            nc.vector.memset(eps_t[:], EPS)
            # per-c-chunk scale/bias vectors: [128, NC] layout, col c = chunk c
            ln1w_s = cpool.tile([128, NC_DIM], F32)
            ln1b_s = cpool.tile([128, NC_DIM], F32)
            ln2w_s = cpool.tile([128, NC_DIM], F32)
            ln2b_s = cpool.tile([128, NC_DIM], F32)
            bqk_s = cpool.tile([128, 2 * NC_DIM], F32)
            bfc1_s = cpool.tile([128, NC_FF], F32)
            nc.sync.dma_start(out=ln1w_s[:], in_=ln1_w.rearrange("(a b) -> b a", b=128))
            nc.sync.dma_start(out=ln1b_s[:], in_=ln1_b.rearrange("(a b) -> b a", b=128))
            nc.sync.dma_start(out=ln2w_s[:], in_=ln2_w.rearrange("(a b) -> b a", b=128))
            nc.sync.dma_start(out=ln2b_s[:], in_=ln2_b.rearrange("(a b) -> b a", b=128))
            nc.sync.dma_start(out=bqk_s[:], in_=bqk.rearrange("(a b) -> b a", b=128))
            nc.sync.dma_start(out=bfc1_s[:], in_=bfc1.rearrange("(a b) -> b a", b=128))
            # partition-broadcast bias rows for token-major epilogues
            bv_bc = cpool.tile([128, DIM], F32)
            bproj_bc = cpool.tile([128, DIM], F32)
            bfc2_bc = cpool.tile([128, DIM], F32)
            nc.sync.dma_start(out=bv_bc[:], in_=bv.unsqueeze(0).partition_broadcast(128))
            nc.sync.dma_start(out=bproj_bc[:], in_=bproj.unsqueeze(0).partition_broadcast(128))
            nc.sync.dma_start(out=bfc2_bc[:], in_=bfc2.unsqueeze(0).partition_broadcast(128))

            # ================= Phase A: LN1 -> xnT =================
            with (
                tc.tile_pool(name="pa", bufs=3) as pa,
                tc.tile_pool(name="pa_s", bufs=8) as pas,
                tc.tile_pool(name="pa_ps", bufs=4, space="PSUM") as paps,
            ):
                for it in range(NT_H):
                    x_t = pa.tile([128, DIM], F32, tag="x_t")
                    nc.sync.dma_start(out=x_t[:], in_=x_loc[it * 128:(it + 1) * 128, :])
                    rstd, nmr = _layernorm_tile(nc, pa, x_t, eps_t)
                    x_hat = pa.tile([128, DIM], F32, tag="x_hat")
                    nc.scalar.activation(out=x_hat[:], in_=x_t[:],
                                         func=mybir.ActivationFunctionType.Identity,
                                         bias=nmr[:], scale=rstd[:])
                    for c in range(NC_DIM):
                        ps = paps.tile([128, 128], F32, tag="tp")
                        nc.tensor.transpose(ps[:], x_hat[:, c * 128:(c + 1) * 128], ident[:])
                        xnT_s = pas.tile([128, 128], F32, tag="xnT_s")
                        nc.scalar.activation(out=xnT_s[:], in_=ps[:],
                                             func=mybir.ActivationFunctionType.Identity,
                                             bias=ln1b_s[:, c:c + 1], scale=ln1w_s[:, c:c + 1])
                        nc.sync.dma_start(
                            out=d_xnT[c * 128:(c + 1) * 128, it * 128:(it + 1) * 128],
                            in_=xnT_s[:])

            # ================= Phase B: qkv =================
            with (
                tc.tile_pool(name="pb_xn", bufs=1) as pbx,
                tc.tile_pool(name="pb_w", bufs=3) as pbw,
                tc.tile_pool(name="pb_s", bufs=4) as pbs,
                tc.tile_pool(name="pb_ps", bufs=4, space="PSUM") as pbps,
            ):
                xn_sb = pbx.tile([128, NC_DIM, TOKH], F32)
                for c in range(NC_DIM):
                    nc.sync.dma_start(out=xn_sb[:, c, :], in_=d_xnT[c * 128:(c + 1) * 128, :])

                # q + k (transposed outputs)
                for oc in range(2 * NC_DIM):  # 0..7 q, 8..15 k
                    is_q = oc < NC_DIM
                    wt = pbw.tile([128, NC_DIM, 128], F32, tag="wqk_t")
                    for c in range(NC_DIM):
                        nc.sync.dma_start(
                            out=wt[:, c, :],
                            in_=wqkT[c * 128:(c + 1) * 128, oc * 128:(oc + 1) * 128])
                    t_end = TOK if is_q else TOKH
                    nt = (t_end + 511) // 512
                    for tcn in range(nt):
                        t0 = tcn * 512
                        w = min(512, t_end - t0)
                        ps = pbps.tile([128, 512], F32, tag="qk_ps")
                        for c in range(NC_DIM):
                            nc.tensor.matmul(
                                _mm_cast(ps[:, :w], F32),
                                lhsT=_mm_cast(wt[:, c, :], MM_BIG),
                                rhs=_mm_cast(xn_sb[:, c, t0:t0 + w], MM_BIG),
                                start=(c == 0), stop=(c == NC_DIM - 1))
                        o_sb = pbs.tile([128, 512], F32, tag="qk_o")
                        nc.scalar.activation(out=o_sb[:, :w], in_=ps[:, :w],
                                             func=mybir.ActivationFunctionType.Identity,
                                             bias=bqk_s[:, oc:oc + 1], scale=1.0)
                        dst = d_qT if is_q else d_kT
                        o0 = (oc if is_q else oc - NC_DIM) * 128
                        nc.sync.dma_start(out=dst[o0:o0 + 128, t0:t0 + w],
                                          in_=o_sb[:, :w])

                # v (token-major)
                wv_sb = pbx.tile([128, NC_DIM, DIM], F32)
                for c in range(NC_DIM):
                    nc.sync.dma_start(out=wv_sb[:, c, :], in_=wvT[c * 128:(c + 1) * 128, :])
                for it in range(NT_H):
                    for oc in range(2):
                        ps = pbps.tile([128, 512], F32, tag="v_ps")
                        for c in range(NC_DIM):
                            nc.tensor.matmul(
                                ps[:],
                                lhsT=_mm_cast(xn_sb[:, c, it * 128:(it + 1) * 128], MM_BIG),
                                rhs=_mm_cast(wv_sb[:, c, oc * 512:(oc + 1) * 512], MM_BIG),
                                start=(c == 0), stop=(c == NC_DIM - 1))
                        v_sb = pbs.tile([128, 512], F32, tag="v_o")
                        nc.vector.tensor_add(out=v_sb[:], in0=ps[:],
                                             in1=bv_bc[:, oc * 512:(oc + 1) * 512])
                        nc.sync.dma_start(
                            out=d_v[it * 128:(it + 1) * 128, oc * 512:(oc + 1) * 512],
                            in_=v_sb[:])

            # ================= Phase C: attention =================
            with (
                tc.tile_pool(name="pc_io", bufs=3) as pcio,
                tc.tile_pool(name="pc_s", bufs=6) as pcs,
                tc.tile_pool(name="pc_st", bufs=8) as pcst,
                tc.tile_pool(name="pc_ps", bufs=2, space="PSUM") as pcps,
                tc.tile_pool(name="pc_ps2", bufs=2, space="PSUM") as pcps2,
                tc.tile_pool(name="pc_ps3", bufs=2, space="PSUM") as pcps3,
            ):
                for j in range(NBLK):
                    lo = 0 if j == 0 else (j - 1) * WIN
                    hi = (j + 2) * WIN
                    wk = hi - lo            # 256 or 384
                    nck = wk // WIN         # kv chunks: 2 or 3
                    q_sb = pcio.tile([128, NC_DIM, 128], F32, tag="q_sb")
                    k_sb = pcio.tile([128, NC_DIM, 384], F32, tag="k_sb")
                    v_sb = pcio.tile([128, 3, DIM], F32, tag="v_sb")
                    for c in range(NC_DIM):
                        nc.sync.dma_start(out=q_sb[:, c, :],
                                          in_=d_qT[c * 128:(c + 1) * 128, j * WIN:(j + 1) * WIN])
                        nc.sync.dma_start(out=k_sb[:, c, :wk],
                                          in_=d_kT[c * 128:(c + 1) * 128, lo:hi])
                    for kc in range(nck):
                        nc.sync.dma_start(out=v_sb[:, kc, :],
                                          in_=d_v[lo + kc * 128:lo + (kc + 1) * 128, :])
                    for h in range(HEADS):
                        hc, hp = h // 2, (h % 2) * 64
                        sim_ps = pcps.tile([128, 384], F32, tag="sim")
                        nc.tensor.matmul(
                            _mm_cast(sim_ps[:, :wk], F32),
                            lhsT=_mm_cast(q_sb[hp:hp + 64, hc, :], MM_ATT),
                            rhs=_mm_cast(k_sb[hp:hp + 64, hc, :wk], MM_ATT),
                            start=True, stop=True)
                        negmax = pcst.tile([128, 1], F32, tag="negmax")
                        nc.vector.reduce_max(out=negmax[:], in_=sim_ps[:, :wk],
                                             axis=mybir.AxisListType.X, negate=True)
                        probs = pcs.tile([128, 384], F32, tag="probs")
                        rsum = pcst.tile([128, 1], F32, tag="rsum")
                        nc.scalar.activation(out=probs[:, :wk], in_=sim_ps[:, :wk],
                                             func=mybir.ActivationFunctionType.Exp,
                                             bias=negmax[:], scale=1.0,
                                             accum_out=rsum[:])
                        rinv = pcst.tile([128, 1], F32, tag="rinv")
                        nc.vector.reciprocal(out=rinv[:], in_=rsum[:])
                        nc.vector.tensor_scalar_mul(probs[:, :wk], in0=probs[:, :wk],
                                                    scalar1=rinv[:])
                        att_ps = pcps3.tile([64, 128], F32, tag="att")
                        for kc in range(nck):
                            pt_ps = pcps2.tile([128, 128], F32, tag="ptp")
                            nc.tensor.transpose(
                                pt_ps[:], probs[:, kc * 128:(kc + 1) * 128], ident[:])
                            pT_sb = pcs.tile([128, 128], F32, tag="pT")
                            nc.scalar.copy(out=pT_sb[:], in_=pt_ps[:])
                            nc.tensor.matmul(
                                _mm_cast(att_ps[:], F32),
                                lhsT=_mm_cast(v_sb[:, kc, h * HD:(h + 1) * HD], MM_ATT),
                                rhs=_mm_cast(pT_sb[:], MM_ATT),
                                start=(kc == 0), stop=(kc == nck - 1))
                        ao_sb = pcs.tile([64, 128], F32, tag="ao")
                        nc.scalar.copy(out=ao_sb[:], in_=att_ps[:])
                        nc.sync.dma_start(
                            out=d_attnT[h * HD:(h + 1) * HD, j * WIN:(j + 1) * WIN],
                            in_=ao_sb[:])

            # ============ Phase D: proj + residual + LN2 -> x1, x1nT ============
            with (
                tc.tile_pool(name="pd_w", bufs=1) as pdw,
                tc.tile_pool(name="pd", bufs=3) as pd,
                tc.tile_pool(name="pd_s", bufs=8) as pds,
                tc.tile_pool(name="pd_ps", bufs=4, space="PSUM") as pdps,
            ):
                wp_sb = pdw.tile([128, NC_DIM, DIM], F32)
                for c in range(NC_DIM):
                    nc.sync.dma_start(out=wp_sb[:, c, :], in_=wprojT[c * 128:(c + 1) * 128, :])
                for it in range(NT):
                    a_sb = pd.tile([128, NC_DIM, 128], F32, tag="a_sb")
                    for c in range(NC_DIM):
                        nc.sync.dma_start(out=a_sb[:, c, :],
                                          in_=d_attnT[c * 128:(c + 1) * 128, it * 128:(it + 1) * 128])
                    x_sb = pd.tile([128, DIM], F32, tag="x_sb")
                    nc.sync.dma_start(out=x_sb[:], in_=x_loc[it * 128:(it + 1) * 128, :])
                    x1_sb = pd.tile([128, DIM], F32, tag="x1_sb")
                    for oc in range(2):
                        ps = pdps.tile([128, 512], F32, tag="proj_ps")
                        for c in range(NC_DIM):
                            nc.tensor.matmul(
                                ps[:],
                                lhsT=_mm_cast(a_sb[:, c, :], MM_BIG),
                                rhs=_mm_cast(wp_sb[:, c, oc * 512:(oc + 1) * 512], MM_BIG),
                                start=(c == 0), stop=(c == NC_DIM - 1))
                        sl = slice(oc * 512, (oc + 1) * 512)
                        nc.vector.tensor_add(out=x1_sb[:, sl], in0=ps[:], in1=x_sb[:, sl])
                        nc.vector.tensor_add(out=x1_sb[:, sl], in0=x1_sb[:, sl],
                                             in1=bproj_bc[:, sl])
                    nc.sync.dma_start(out=d_x1[it * 128:(it + 1) * 128, :], in_=x1_sb[:])
                    # LN2 + transpose
                    rstd, nmr = _layernorm_tile(nc, pd, x1_sb, eps_t)
                    x1h = pd.tile([128, DIM], F32, tag="x1h")
                    nc.scalar.activation(out=x1h[:], in_=x1_sb[:],
                                         func=mybir.ActivationFunctionType.Identity,
                                         bias=nmr[:], scale=rstd[:])
                    for c in range(NC_DIM):
                        ps = pdps.tile([128, 128], F32, tag="tp2")
                        nc.tensor.transpose(ps[:], x1h[:, c * 128:(c + 1) * 128], ident[:])
                        xnT_s = pds.tile([128, 128], F32, tag="x1nT_s")
                        nc.scalar.activation(out=xnT_s[:], in_=ps[:],
                                             func=mybir.ActivationFunctionType.Identity,
                                             bias=ln2b_s[:, c:c + 1], scale=ln2w_s[:, c:c + 1])
                        nc.sync.dma_start(
                            out=d_x1nT[c * 128:(c + 1) * 128, it * 128:(it + 1) * 128],
                            in_=xnT_s[:])

            # ================= Phase E: fc1 + gelu -> gT =================
            with (
                tc.tile_pool(name="pe_xn", bufs=1) as pex,
                tc.tile_pool(name="pe_w", bufs=3) as pew,
                tc.tile_pool(name="pe_s", bufs=4) as pes,
                tc.tile_pool(name="pe_ps", bufs=4, space="PSUM") as peps,
            ):
                x1n_sb = pex.tile([128, NC_DIM, TOK], F32)
                for c in range(NC_DIM):
                    nc.sync.dma_start(out=x1n_sb[:, c, :], in_=d_x1nT[c * 128:(c + 1) * 128, :])
                for fc in range(NC_FF):
                    wt = pew.tile([128, NC_DIM, 128], F32, tag="w1_t")
                    for c in range(NC_DIM):
                        nc.sync.dma_start(
                            out=wt[:, c, :],
                            in_=wfc1T[c * 128:(c + 1) * 128, fc * 128:(fc + 1) * 128])
                    for tcn in range(TOK // 512):
                        t0 = tcn * 512
                        ps = peps.tile([128, 512], F32, tag="fc1_ps")
                        for c in range(NC_DIM):
                            nc.tensor.matmul(
                                ps[:],
                                lhsT=_mm_cast(wt[:, c, :], MM_BIG),
                                rhs=_mm_cast(x1n_sb[:, c, t0:t0 + 512], MM_BIG),
                                start=(c == 0), stop=(c == NC_DIM - 1))
                        g_sb = pes.tile([128, 512], F32, tag="g_o")
                        nc.scalar.activation(out=g_sb[:], in_=ps[:],
                                             func=mybir.ActivationFunctionType.Gelu,
                                             bias=bfc1_s[:, fc:fc + 1], scale=1.0)
                        nc.sync.dma_start(
                            out=d_gT[fc * 128:(fc + 1) * 128, t0:t0 + 512],
                            in_=g_sb[:])

            # ================= Phase F: fc2 + residual -> out =================
            with (
                tc.tile_pool(name="pf_w", bufs=1) as pfw,
                tc.tile_pool(name="pf", bufs=2) as pf,
                tc.tile_pool(name="pf_s", bufs=4) as pfs,
                tc.tile_pool(name="pf_ps", bufs=4, space="PSUM") as pfps,
            ):
                w2_sb = pfw.tile([128, NC_FF, DIM], F32)
                for fc in range(NC_FF):
                    nc.sync.dma_start(out=w2_sb[:, fc, :], in_=wfc2T[fc * 128:(fc + 1) * 128, :])
                for it in range(NT):
                    g_sb = pf.tile([128, NC_FF, 128], F32, tag="g_sb")
                    for fc in range(NC_FF):
                        nc.sync.dma_start(out=g_sb[:, fc, :],
                                          in_=d_gT[fc * 128:(fc + 1) * 128, it * 128:(it + 1) * 128])
                    x1_sb = pf.tile([128, DIM], F32, tag="x1r")
                    nc.sync.dma_start(out=x1_sb[:], in_=d_x1[it * 128:(it + 1) * 128, :])
                    o_sb = pfs.tile([128, DIM], F32, tag="o_sb")
                    for oc in range(2):
                        ps = pfps.tile([128, 512], F32, tag="fc2_ps")
                        for fc in range(NC_FF):
                            nc.tensor.matmul(
                                ps[:],
                                lhsT=_mm_cast(g_sb[:, fc, :], MM_BIG),
                                rhs=_mm_cast(w2_sb[:, fc, oc * 512:(oc + 1) * 512], MM_BIG),
                                start=(fc == 0), stop=(fc == NC_FF - 1))
                        sl = slice(oc * 512, (oc + 1) * 512)
                        nc.vector.tensor_add(out=o_sb[:, sl], in0=ps[:], in1=x1_sb[:, sl])
                        nc.vector.tensor_add(out=o_sb[:, sl], in0=o_sb[:, sl],
                                             in1=bfc2_bc[:, sl])
                    nc.sync.dma_start(out=out_loc[it * 128:(it + 1) * 128, :], in_=o_sb[:])

    nc.compile()
    return nc


_NC_CACHE = None


def _get_program():
    global _NC_CACHE
    if _NC_CACHE is None:
        _NC_CACHE = _build_program()
    return _NC_CACHE


def _prep_inputs(inputs):
    x = np.ascontiguousarray(np.asarray(inputs["x"], np.float32))
    qkv_w = np.asarray(inputs["qkv_w"], np.float32)
    qkv_b = np.asarray(inputs["qkv_b"], np.float32)
    wq = qkv_w[0:DIM] * SCALE
    wk = qkv_w[DIM:2 * DIM]
    wv = qkv_w[2 * DIM:]
    shared = {
        "ln1_w": np.ascontiguousarray(inputs["ln1_w"], np.float32),
        "ln1_b": np.ascontiguousarray(inputs["ln1_b"], np.float32),
        "ln2_w": np.ascontiguousarray(inputs["ln2_w"], np.float32),
        "ln2_b": np.ascontiguousarray(inputs["ln2_b"], np.float32),
        "wqkT": np.ascontiguousarray(np.concatenate([wq, wk], 0).T),
        "bqk": np.ascontiguousarray(
            np.concatenate([qkv_b[0:DIM] * SCALE, qkv_b[DIM:2 * DIM]], 0)),
        "wvT": np.ascontiguousarray(wv.T),
        "bv": np.ascontiguousarray(qkv_b[2 * DIM:]),
        "wprojT": np.ascontiguousarray(np.asarray(inputs["proj_w"], np.float32).T),
        "bproj": np.ascontiguousarray(inputs["proj_b"], np.float32),
        "wfc1T": np.ascontiguousarray(np.asarray(inputs["fc1_w"], np.float32).T),
        "bfc1": np.ascontiguousarray(inputs["fc1_b"], np.float32),
        "wfc2T": np.ascontiguousarray(np.asarray(inputs["fc2_w"], np.float32).T),
        "bfc2": np.ascontiguousarray(inputs["fc2_b"], np.float32),
    }
    in_maps = []
    for c in range(NCORES):
        b, half = c // 2, c % 2
        seq = x[b]
        if half == 0:
            loc = seq[0:TOKH]
        else:
            loc = seq[N - TOKH:][::-1]  # loc[t] = seq[N-1-t]
        m = dict(shared)
        m["x_loc"] = np.ascontiguousarray(loc)
        in_maps.append(m)
    return in_maps


def kernel(**inputs):
    nc = _get_program()
    in_maps = _prep_inputs(inputs)
    res = bass_utils.run_bass_kernel_spmd(nc, in_maps, core_ids=list(range(NCORES)))
    out = np.zeros((B, N, DIM), np.float32)
    for c in range(NCORES):
        b, half = c // 2, c % 2
        o = res.results[c]["out_loc"]
        if half == 0:
            out[b, 0:TOK] = o
        else:
            out[b, TOK:] = o[::-1]
    return out
